# revision 21
# baseline (speedup 1.0000x reference)
"""BitLinear (int8-activation x ternary-weight) matmul on 8 TRN2 NeuronCores.

Full inputs: x [4, 4096, 2048] f32, weight [2048, 2048] f32.
Output: [4, 4096, 2048] fp16 = ((qx @ qw.T) / si / sw).astype(f16).

Strategy: data-parallel over the 16384 rows (2048 rows/core). The weight
is replicated; each core computes mean|W| on-device (first W read),
then quantizes W to ternary {-1,0,1} stored as fp8. Per-row activation
quantization to int8 values held in bf16 uses the fp32 magic-number
trick (v + 1.5*2^23 rounds to the nearest integer, RNE). The matmul
runs bf16(lhsT=qx^T) x fp8(qw^T) on the TensorEngine with fp32 PSUM
accumulation -- exact for these integer values -- and the dequant
(acc * amax/127 * mean|W|) is fused into the PSUM->SBUF fp16 copy on
the ScalarEngine.

The serial head is the 16MB W read: the mean needs every element, and
the head is DMA-descriptor/bandwidth-bound, so everything else is kept
off the queues during it. x loads and output stores issue from the
scalar ring (sync ring is W-only); the first two row tiles' x^T
transposes run on the otherwise-idle PE (identity-ifmap transpose into
PSUM, DVE copy out) instead of the DMA xbar, whose 32x32-tile
descriptors would stall the W stream; the W re-reads (non-cached
k-tiles) are allocated out of the cache pool so each re-read DMA is
dependency-gated behind the requantize wave's consumption of a cached
tile and streams in the post-head bandwidth hole. The wave itself is
one DVE op (u = w*sw + MAGIC) plus one ACT Sign op per k-tile
(sign(n) == clip(n,-1,1) for integers), with the first two row tiles'
matmuls interleaved per k-tile and row tiles 2/3 prepped in the wave's
DVE slack (their xbar transposes ride the idle post-head queues). Junk
matmuls keep the PE clock warm through the head. Host only
reshapes/shards, transposes W, and supplies an identity tile (layout
prep, no math).
"""

import numpy as np

import concourse.mybir as mybir
import concourse.tile as tile
from concourse import bacc
from concourse.bass import ts
from concourse.bass_utils import run_bass_kernel_spmd

N_CORES = 8
ROWS_TOTAL = 4 * 4096
K = 2048
N = 2048
NCACHE = 16  # all W k-tiles cached in SBUF: no re-read traffic at all
MAGIC = 12582912.0  # 1.5*2^23: fp32 round-to-nearest-even (both signs)

f32 = mybir.dt.float32
bf16 = mybir.dt.bfloat16
f16 = mybir.dt.float16
fp8 = mybir.dt.float8e4
Alu = mybir.AluOpType
Act = mybir.ActivationFunctionType
AxX = mybir.AxisListType.X


def build(rows_per_core=ROWS_TOTAL // N_CORES):
    nc = bacc.Bacc(
        "TRN2", target_bir_lowering=False, debug=False, num_devices=N_CORES
    )
    x_ext = nc.declare_dram_parameter("x", [rows_per_core, K], f32, isOutput=False)
    wt_ext = nc.declare_dram_parameter("wt", [K, N], f32, isOutput=False)
    id_ext = nc.declare_dram_parameter("ident", [128, 128], bf16, isOutput=False)
    out_ext = nc.declare_dram_parameter(
        "out", [rows_per_core, N], f16, isOutput=True
    )

    KT = K // 128
    MT = rows_per_core // 128
    NQ = N // 512

    with tile.TileContext(nc) as tc:
        with (
            tc.tile_pool(name="xin", bufs=2) as xin,  # [128,K] f32 x loads
            tc.tile_pool(name="wch", bufs=NCACHE) as wch,  # cached W tiles
            tc.tile_pool(name="qtmp", bufs=1) as qtmp,  # qx bf16
            tc.tile_pool(name="qxt", bufs=4) as qxtp,  # [128,KT,128] bf16 x^T
            tc.tile_pool(name="outp", bufs=1) as outp,  # [128,N] f16 results
            tc.tile_pool(name="singles", bufs=1) as singles,
            tc.tile_pool(name="small", bufs=6) as small,  # [128,1] stats
            tc.tile_pool(name="pacc", bufs=8, space="PSUM") as pacc,
        ):
            ones_mat = singles.tile([128, 128], f32)
            nc.vector.memset(ones_mat, 1.0)
            qwT = singles.tile([128, KT, N], fp8)
            wsums = singles.tile([128, KT], f32)
            negm = singles.tile([128, 1], f32)  # bias AP for Sign (-MAGIC)
            nc.vector.memset(negm, -MAGIC)
            ident = singles.tile([128, 128], bf16)
            nc.scalar.dma_start(out=ident, in_=id_ext[:, :])

            def x_load(mi, ring=None):
                x_t = xin.tile([128, K], f32, tag="xin", name=f"x{mi}")
                (ring or nc.scalar).dma_start(out=x_t, in_=x_ext[ts(mi, 128), :])
                return x_t

            def x_quant(mi, pe_transpose=False, transpose_ring=None,
                        xs_on_dve=False, load_ring=None):
                x_t = x_pre.pop(mi, None)
                if x_t is None:
                    x_t = x_load(mi, ring=load_ring)
                amax = small.tile([128, 1], f32, tag="small")
                nc.vector.tensor_reduce(
                    out=amax, in_=x_t, axis=AxX, op=Alu.max,
                    apply_absolute_value=True,
                )
                amc = small.tile([128, 1], f32, tag="amc", name=f"amc{mi}")
                nc.vector.tensor_scalar_max(out=amc, in0=amax, scalar1=1e-5)
                rec = small.tile([128, 1], f32, tag="small")
                nc.vector.reciprocal(out=rec, in_=amc)
                si = small.tile([128, 1], f32, tag="small")
                nc.vector.tensor_scalar_mul(out=si, in0=rec, scalar1=127.0)
                # u = x*si + MAGIC in place (rounds to integer, RNE); x's
                # raw values are dead once amax is computed
                nc.vector.tensor_scalar(
                    out=x_t, in0=x_t, scalar1=si, scalar2=MAGIC,
                    op0=Alu.mult, op1=Alu.add,
                )
                qx = qtmp.tile([128, K], bf16, tag="qtmp")
                nc.vector.tensor_scalar(
                    out=qx, in0=x_t, scalar1=-MAGIC, scalar2=None, op0=Alu.add,
                )
                qxT = qxtp.tile(
                    [128, KT, 128], bf16, tag="qxt", name=f"qxT{mi}"
                )
                if pe_transpose:
                    # PE-side transpose: zero DMA-queue cost; runs in the
                    # head where the PE is otherwise doing junk warmups.
                    half = KT // 2
                    for h in range(2):
                        pt = pacc.tile(
                            [128, 512], f32, tag="acc", name=f"pt{mi}_{h}"
                        )
                        ptb = pt.bitcast(bf16)  # [128, 1024]
                        for j in range(half):
                            kt = h * half + j
                            nc.tensor.transpose(
                                ptb[:, ts(j, 128)],
                                qx[:, ts(kt, 128)],
                                ident,
                            )
                        nc.vector.tensor_copy(
                            out=qxT[:, h * half : (h + 1) * half, :], in_=ptb
                        )
                else:
                    ring = transpose_ring or nc.scalar
                    ring.dma_start_transpose(out=qxT, in_=qx)
                return qxT, amc

            # ---- PE warm-up: the HAM clock gate halves the PE clock after
            # ~3.4us idle, and the PE has no real work until quantized W
            # tiles arrive. Junk matmuls rotating through the pacc slots
            # hold the clock up through the W-prep head; they are emitted
            # in chunks around the first two x tiles' PE transposes.
            warm_src = singles.tile([128, 512], bf16)
            nc.vector.memset(warm_src, 1.0)
            wi = 0

            def warm(n):
                nonlocal wi
                for _ in range(n):
                    pwarm = pacc.tile(
                        [128, 512], f32, tag="acc", name=f"warm{wi}"
                    )
                    nc.tensor.matmul(
                        pwarm, lhsT=warm_src[:, :128], rhs=warm_src,
                        start=True, stop=True, skip_group_check=True,
                    )
                    wi += 1

            # ---- W pass 1: mean(|W|); k-tiles 0..NCACHE-1 live in the
            # cache pool, the rest stream through wld. The first two x
            # tiles load via the scalar ring and their quantize chains are
            # emitted between the W reduces.
            x_pre = {}
            xq = {}
            if MT >= 1:
                x_pre[0] = x_load(0)
            if MT >= 2:
                x_pre[1] = x_load(1)
            warm(215)

            def x_load_gated(mi):
                # a tiny ACT write into the tile holds the x DMA (WAW) until
                # the in-order ACT Abs train reaches this point, i.e. until
                # W k-tile arrivals have drained past it -- so the x load
                # rides the W-tail's idle bandwidth instead of competing
                # with the head of the stream
                x_t = xin.tile([128, K], f32, tag="xin", name=f"x{mi}")
                nc.scalar.activation(out=x_t[:, 0:1], in_=negm, func=Act.Copy)
                nc.scalar.dma_start(out=x_t, in_=x_ext[ts(mi, 128), :])
                x_pre[mi] = x_t

            wcache_tiles = {}
            for kt in range(KT):
                if kt < NCACHE:
                    wt_t = wch.tile([128, K], f32, tag="wch", name=f"wch{kt}")
                    wcache_tiles[kt] = wt_t
                else:
                    wt_t = wld.tile([128, K], f32, tag="wld", name=f"wld{kt}")
                nc.sync.dma_start(out=wt_t, in_=wt_ext[ts(kt, 128), :])
                # |w| row-sums on the otherwise-idle ACT engine (accum_out);
                # the Abs output itself is trash, written over qwT space
                # that the wave overwrites later (bf16 view of 2 fp8 slots)
                tk = min(kt, KT - 2)
                trash = qwT[:, tk : tk + 2, :].bitcast(bf16)
                nc.scalar.activation(
                    out=trash, in_=wt_t, func=Act.Abs,
                    accum_out=wsums[:, kt : kt + 1],
                )

            xq[0] = x_quant(0, pe_transpose=True, xs_on_dve=True)
            if MT >= 2:
                xq[1] = x_quant(1, pe_transpose=True, xs_on_dve=True)
            wtot = small.tile([128, 1], f32, tag="small")
            nc.vector.tensor_reduce(out=wtot, in_=wsums, axis=AxX, op=Alu.add)
            # ones_mat.T @ wtot replicates the grand total across all 128
            # partitions in one matmul, so the scale math runs as [128,1]
            # vectors with no extra broadcast round-trips
            ptot_b = pacc.tile([128, 1], f32, tag="acc", name="ptot_b")
            nc.tensor.matmul(ptot_b, lhsT=ones_mat, rhs=wtot, start=True, stop=True)
            # meanc = max(mean|W|, 1e-5); sw = 1/meanc; q = meanc/127
            meanc_b = small.tile([128, 1], f32, tag="s1")
            nc.vector.tensor_scalar(
                out=meanc_b,
                in0=ptot_b,
                scalar1=1.0 / (K * N),
                scalar2=1e-5,
                op0=Alu.mult,
                op1=Alu.max,
            )
            sw_b = singles.tile([128, 1], f32)
            nc.vector.reciprocal(out=sw_b, in_=meanc_b)
            q_b = singles.tile([128, 1], f32)
            nc.vector.tensor_scalar_mul(out=q_b, in0=meanc_b, scalar1=1.0 / 127.0)

            wreread_tiles = {}

            # ---- main loop helpers
            def mm(acc, qxT, kt, nq):
                nc.tensor.matmul(
                    acc, lhsT=qxT[:, kt, :], rhs=qwT[:, kt, ts(nq, 512)],
                    start=(kt == 0), stop=(kt == KT - 1),
                    skip_group_check=True,
                )

            def finish(mi, accs, amc):
                cs = small.tile([128, 1], f32, tag="small")
                nc.vector.tensor_mul(cs, amc, q_b)  # (amax/127)*meanc
                o_t = outp.tile([128, N], f16, tag="outp", name=f"o{mi}")
                for nq in range(NQ):
                    nc.scalar.activation(
                        out=o_t[:, ts(nq, 512)], in_=accs[nq],
                        func=Act.Copy, scale=cs,
                    )
                nc.scalar.dma_start(out=out_ext[ts(mi, 128), :], in_=o_t)

            def quantize_w(kt):
                wt_t = wcache_tiles.get(kt) or wreread_tiles.get(kt)
                nc.vector.tensor_scalar(
                    out=wt_t, in0=wt_t, scalar1=sw_b, scalar2=MAGIC,
                    op0=Alu.mult, op1=Alu.add,
                )
                nc.scalar.activation(
                    out=qwT[:, kt, :], in_=wt_t, func=Act.Sign, bias=negm
                )

            # ---- W pass 2 (the wave): qwT[kt] = sign(round(wT*sw)) as fp8,
            # one DVE op (u = w*sw + MAGIC, rounds to integer) plus one ACT
            # op. The first two row tiles' matmuls interleave with the wave
            # so each arriving qwT k-tile immediately unlocks 8 matmuls,
            # and x2/x3's quantize chains ride the wave's DVE slack (their
            # transposes issue from the now-idle sync ring).
            if MT >= 2:
                qxT0, amc0 = xq[0]
                qxT1, amc1 = xq[1]
                accs0 = [
                    pacc.tile([128, 512], f32, tag="acc", name=f"acc_0_{i}")
                    for i in range(NQ)
                ]
                accs1 = [
                    pacc.tile([128, 512], f32, tag="acc", name=f"acc_1_{i}")
                    for i in range(NQ)
                ]
                for kt in range(KT):
                    quantize_w(kt)
                    for nq in range(NQ):
                        mm(accs0[nq], qxT0, kt, nq)
                    for nq in range(NQ):
                        mm(accs1[nq], qxT1, kt, nq)
                    if kt == 2 and MT > 2:
                        x_load_gated(2)
                    if kt == 4 and MT > 2:
                        xq[2] = x_quant(2, transpose_ring=nc.sync,
                                        xs_on_dve=True)
                    if kt == 5 and MT > 3:
                        x_load_gated(3)
                    if kt == 7 and MT > 3:
                        xq[3] = x_quant(3, transpose_ring=nc.sync,
                                        xs_on_dve=True)
                finish(0, accs0, amc0)
                finish(1, accs1, amc1)
                start_mi = 2
            else:
                for kt in range(KT):
                    quantize_w(kt)
                start_mi = 0

            # ---- dense phase over the remaining row tiles
            for mi in range(start_mi, MT):
                if mi in xq:
                    qxT, amc = xq[mi]
                else:
                    qxT, amc = x_quant(mi)
                accs = [
                    pacc.tile([128, 512], f32, tag="acc", name=f"acc_{mi}_{i}")
                    for i in range(NQ)
                ]
                if mi == MT - 1:
                    # nq-inner with per-chunk dequant+store: each output
                    # chunk is dequantized and written out as soon as its
                    # 16 accumulations are done (shortest possible tail)
                    cs = small.tile([128, 1], f32, tag="small")
                    nc.vector.tensor_mul(cs, amc, q_b)
                    o_t = outp.tile([128, N], f16, tag="outp", name=f"o{mi}")
                    for nq in range(NQ):
                        for kt in range(KT):
                            mm(accs[nq], qxT, kt, nq)
                        nc.scalar.activation(
                            out=o_t[:, ts(nq, 512)], in_=accs[nq],
                            func=Act.Copy, scale=cs,
                        )
                        nc.scalar.dma_start(
                            out=out_ext[ts(mi, 128), ts(nq, 512)],
                            in_=o_t[:, ts(nq, 512)],
                        )
                else:
                    for kt in range(KT):
                        for nq in range(NQ):
                            mm(accs[nq], qxT, kt, nq)
                    finish(mi, accs, amc)

    nc.compile()
    return nc


_NC_CACHE = {}


def _get_nc(rows_per_core):
    if rows_per_core not in _NC_CACHE:
        _NC_CACHE[rows_per_core] = build(rows_per_core)
    return _NC_CACHE[rows_per_core]


_IDENT = np.eye(128, dtype=np.float32)


def _ident_bf16():
    # bf16 identity as uint16 view (ml_dtypes-free): 1.0 -> 0x3F80
    e = np.zeros((128, 128), dtype=np.uint16)
    np.fill_diagonal(e, 0x3F80)
    return e


def run(x, weight, **spmd_kwargs):
    x = np.ascontiguousarray(np.asarray(x, dtype=np.float32))
    weight = np.asarray(weight, dtype=np.float32)
    b, s, k = x.shape
    rows = b * s
    rpc = rows // N_CORES
    xr = x.reshape(rows, k)
    wt = np.ascontiguousarray(weight.T)
    ident = _ident_bf16()
    nc = _get_nc(rpc)
    in_maps = [
        {"x": xr[i * rpc : (i + 1) * rpc], "wt": wt, "ident": ident}
        for i in range(N_CORES)
    ]
    res = run_bass_kernel_spmd(
        nc, in_maps, core_ids=list(range(N_CORES)), **spmd_kwargs
    )
    out = np.concatenate(
        [res.results[i]["out"] for i in range(N_CORES)], axis=0
    )
    return out.reshape(b, s, N), res


def kernel(x, weight):
    out, _ = run(x, weight)
    return out


# revision 22
# speedup vs baseline: 1.0272x; 1.0272x over previous
"""BitLinear (int8-activation x ternary-weight) matmul on 8 TRN2 NeuronCores.

Full inputs: x [4, 4096, 2048] f32, weight [2048, 2048] f32.
Output: [4, 4096, 2048] fp16 = ((qx @ qw.T) / si / sw).astype(f16).

Strategy: data-parallel over the 16384 rows (2048 rows/core). The weight
is replicated; each core computes mean|W| on-device (first W read),
then quantizes W to ternary {-1,0,1} stored as fp8. Per-row activation
quantization to int8 values held in bf16 uses the fp32 magic-number
trick (v + 1.5*2^23 rounds to the nearest integer, RNE). The matmul
runs bf16(lhsT=qx^T) x fp8(qw^T) on the TensorEngine with fp32 PSUM
accumulation -- exact for these integer values -- and the dequant
(acc * amax/127 * mean|W|) is fused into the PSUM->SBUF fp16 copy on
the ScalarEngine.

The serial head is the 16MB W read: the mean needs every element, and
the head is DMA-descriptor/bandwidth-bound, so everything else is kept
off the queues during it. x loads and output stores issue from the
scalar ring (sync ring is W-only); the first two row tiles' x^T
transposes run on the otherwise-idle PE (identity-ifmap transpose into
PSUM, DVE copy out) instead of the DMA xbar, whose 32x32-tile
descriptors would stall the W stream; the W re-reads (non-cached
k-tiles) are allocated out of the cache pool so each re-read DMA is
dependency-gated behind the requantize wave's consumption of a cached
tile and streams in the post-head bandwidth hole. The wave itself is
one DVE op (u = w*sw + MAGIC) plus one ACT Sign op per k-tile
(sign(n) == clip(n,-1,1) for integers), with the first two row tiles'
matmuls interleaved per k-tile and row tiles 2/3 prepped in the wave's
DVE slack (their xbar transposes ride the idle post-head queues). Junk
matmuls keep the PE clock warm through the head. Host only
reshapes/shards, transposes W, and supplies an identity tile (layout
prep, no math).
"""

import numpy as np

import concourse.mybir as mybir
import concourse.tile as tile
from concourse import bacc
from concourse.bass import ts
from concourse.bass_utils import run_bass_kernel_spmd

N_CORES = 8
ROWS_TOTAL = 4 * 4096
K = 2048
N = 2048
NCACHE = 16  # all W k-tiles cached in SBUF: no re-read traffic at all
MAGIC = 12582912.0  # 1.5*2^23: fp32 round-to-nearest-even (both signs)

f32 = mybir.dt.float32
bf16 = mybir.dt.bfloat16
f16 = mybir.dt.float16
fp8 = mybir.dt.float8e4
Alu = mybir.AluOpType
Act = mybir.ActivationFunctionType
AxX = mybir.AxisListType.X


def build(rows_per_core=ROWS_TOTAL // N_CORES):
    nc = bacc.Bacc(
        "TRN2", target_bir_lowering=False, debug=False, num_devices=N_CORES
    )
    x_ext = nc.declare_dram_parameter("x", [rows_per_core, K], f32, isOutput=False)
    wt_ext = nc.declare_dram_parameter("wt", [K, N], f32, isOutput=False)
    id_ext = nc.declare_dram_parameter("ident", [128, 128], bf16, isOutput=False)
    out_ext = nc.declare_dram_parameter(
        "out", [rows_per_core, N], f16, isOutput=True
    )

    KT = K // 128
    MT = rows_per_core // 128
    NQ = N // 512

    with tile.TileContext(nc) as tc:
        with (
            tc.tile_pool(name="xin", bufs=2) as xin,  # [128,K] f32 x loads
            tc.tile_pool(name="wch", bufs=NCACHE) as wch,  # cached W tiles
            tc.tile_pool(name="qtmp", bufs=1) as qtmp,  # qx bf16
            tc.tile_pool(name="qxt", bufs=4) as qxtp,  # [128,KT,128] bf16 x^T
            tc.tile_pool(name="outp", bufs=1) as outp,  # [128,N] f16 results
            tc.tile_pool(name="singles", bufs=1) as singles,
            tc.tile_pool(name="small", bufs=6) as small,  # [128,1] stats
            tc.tile_pool(name="pacc", bufs=8, space="PSUM") as pacc,
        ):
            ones_mat = singles.tile([128, 128], f32)
            nc.vector.memset(ones_mat, 1.0)
            qwT = singles.tile([128, KT, N], fp8)
            wsums = singles.tile([128, KT], f32)
            negm = singles.tile([128, 1], f32)  # bias AP for Sign (-MAGIC)
            nc.vector.memset(negm, -MAGIC)
            ident = singles.tile([128, 128], bf16)
            nc.scalar.dma_start(out=ident, in_=id_ext[:, :])

            def x_load(mi, ring=None):
                x_t = xin.tile([128, K], f32, tag="xin", name=f"x{mi}")
                (ring or nc.scalar).dma_start(out=x_t, in_=x_ext[ts(mi, 128), :])
                return x_t

            def x_quant(mi, pe_transpose=False, transpose_ring=None,
                        xs_on_dve=False, load_ring=None):
                x_t = x_pre.pop(mi, None)
                if x_t is None:
                    x_t = x_load(mi, ring=load_ring)
                amax = small.tile([128, 1], f32, tag="small")
                nc.vector.tensor_reduce(
                    out=amax, in_=x_t, axis=AxX, op=Alu.max,
                    apply_absolute_value=True,
                )
                amc = small.tile([128, 1], f32, tag="amc", name=f"amc{mi}")
                nc.vector.tensor_scalar_max(out=amc, in0=amax, scalar1=1e-5)
                rec = small.tile([128, 1], f32, tag="small")
                nc.vector.reciprocal(out=rec, in_=amc)
                si = small.tile([128, 1], f32, tag="small")
                nc.vector.tensor_scalar_mul(out=si, in0=rec, scalar1=127.0)
                # u = x*si + MAGIC in place (rounds to integer, RNE); x's
                # raw values are dead once amax is computed
                nc.vector.tensor_scalar(
                    out=x_t, in0=x_t, scalar1=si, scalar2=MAGIC,
                    op0=Alu.mult, op1=Alu.add,
                )
                qx = qtmp.tile([128, K], bf16, tag="qtmp")
                nc.vector.tensor_scalar(
                    out=qx, in0=x_t, scalar1=-MAGIC, scalar2=None, op0=Alu.add,
                )
                qxT = qxtp.tile(
                    [128, KT, 128], bf16, tag="qxt", name=f"qxT{mi}"
                )
                if pe_transpose:
                    # PE-side transpose: zero DMA-queue cost; runs in the
                    # head where the PE is otherwise doing junk warmups.
                    half = KT // 2
                    for h in range(2):
                        pt = pacc.tile(
                            [128, 512], f32, tag="acc", name=f"pt{mi}_{h}"
                        )
                        ptb = pt.bitcast(bf16)  # [128, 1024]
                        for j in range(half):
                            kt = h * half + j
                            nc.tensor.transpose(
                                ptb[:, ts(j, 128)],
                                qx[:, ts(kt, 128)],
                                ident,
                            )
                        nc.vector.tensor_copy(
                            out=qxT[:, h * half : (h + 1) * half, :], in_=ptb
                        )
                else:
                    ring = transpose_ring or nc.scalar
                    ring.dma_start_transpose(out=qxT, in_=qx)
                return qxT, amc

            # ---- PE warm-up: the HAM clock gate halves the PE clock after
            # ~3.4us idle, and the PE has no real work until quantized W
            # tiles arrive. Junk matmuls rotating through the pacc slots
            # hold the clock up through the W-prep head; they are emitted
            # in chunks around the first two x tiles' PE transposes.
            warm_src = singles.tile([128, 512], bf16)
            nc.vector.memset(warm_src, 1.0)
            wi = 0

            def warm(n):
                nonlocal wi
                for _ in range(n):
                    pwarm = pacc.tile(
                        [128, 512], f32, tag="acc", name=f"warm{wi}"
                    )
                    nc.tensor.matmul(
                        pwarm, lhsT=warm_src[:, :128], rhs=warm_src,
                        start=True, stop=True, skip_group_check=True,
                    )
                    wi += 1

            # ---- W pass 1: mean(|W|); k-tiles 0..NCACHE-1 live in the
            # cache pool, the rest stream through wld. The first two x
            # tiles load via the scalar ring and their quantize chains are
            # emitted between the W reduces.
            x_pre = {}
            xq = {}
            warm(215)

            def x_load_gated(mi):
                # a tiny ACT write into the tile holds the x DMA (WAW) until
                # the in-order ACT Abs train reaches this point, i.e. until
                # W k-tile arrivals have drained past it -- so the x load
                # rides the W-tail's idle bandwidth instead of competing
                # with the head of the stream
                x_t = xin.tile([128, K], f32, tag="xin", name=f"x{mi}")
                nc.scalar.activation(out=x_t[:, 0:1], in_=negm, func=Act.Copy)
                nc.scalar.dma_start(out=x_t, in_=x_ext[ts(mi, 128), :])
                x_pre[mi] = x_t

            wcache_tiles = {}
            for kt in range(KT):
                if kt < NCACHE:
                    wt_t = wch.tile([128, K], f32, tag="wch", name=f"wch{kt}")
                    wcache_tiles[kt] = wt_t
                else:
                    wt_t = wld.tile([128, K], f32, tag="wld", name=f"wld{kt}")
                nc.sync.dma_start(out=wt_t, in_=wt_ext[ts(kt, 128), :])
                # |w| row-sums on the otherwise-idle ACT engine (accum_out);
                # the Abs output itself is trash, written over qwT space
                # that the wave overwrites later (bf16 view of 2 fp8 slots)
                tk = min(kt, KT - 2)
                trash = qwT[:, tk : tk + 2, :].bitcast(bf16)
                nc.scalar.activation(
                    out=trash, in_=wt_t, func=Act.Abs,
                    accum_out=wsums[:, kt : kt + 1],
                )
                if kt == 9 and MT >= 1:
                    x_load_gated(0)
                if kt == 10 and MT >= 2:
                    x_load_gated(1)

            xq[0] = x_quant(0, pe_transpose=True, xs_on_dve=True)
            if MT >= 2:
                xq[1] = x_quant(1, pe_transpose=True, xs_on_dve=True)
            wtot = small.tile([128, 1], f32, tag="small")
            nc.vector.tensor_reduce(out=wtot, in_=wsums, axis=AxX, op=Alu.add)
            # ones_mat.T @ wtot replicates the grand total across all 128
            # partitions in one matmul, so the scale math runs as [128,1]
            # vectors with no extra broadcast round-trips
            ptot_b = pacc.tile([128, 1], f32, tag="acc", name="ptot_b")
            nc.tensor.matmul(ptot_b, lhsT=ones_mat, rhs=wtot, start=True, stop=True)
            # meanc = max(mean|W|, 1e-5); sw = 1/meanc; q = meanc/127
            meanc_b = small.tile([128, 1], f32, tag="s1")
            nc.vector.tensor_scalar(
                out=meanc_b,
                in0=ptot_b,
                scalar1=1.0 / (K * N),
                scalar2=1e-5,
                op0=Alu.mult,
                op1=Alu.max,
            )
            sw_b = singles.tile([128, 1], f32)
            nc.vector.reciprocal(out=sw_b, in_=meanc_b)
            q_b = singles.tile([128, 1], f32)
            nc.vector.tensor_scalar_mul(out=q_b, in0=meanc_b, scalar1=1.0 / 127.0)

            wreread_tiles = {}

            # ---- main loop helpers
            def mm(acc, qxT, kt, nq):
                nc.tensor.matmul(
                    acc, lhsT=qxT[:, kt, :], rhs=qwT[:, kt, ts(nq, 512)],
                    start=(kt == 0), stop=(kt == KT - 1),
                    skip_group_check=True,
                )

            def finish(mi, accs, amc):
                cs = small.tile([128, 1], f32, tag="small")
                nc.vector.tensor_mul(cs, amc, q_b)  # (amax/127)*meanc
                o_t = outp.tile([128, N], f16, tag="outp", name=f"o{mi}")
                for nq in range(NQ):
                    nc.scalar.activation(
                        out=o_t[:, ts(nq, 512)], in_=accs[nq],
                        func=Act.Copy, scale=cs,
                    )
                nc.scalar.dma_start(out=out_ext[ts(mi, 128), :], in_=o_t)

            def quantize_w(kt):
                wt_t = wcache_tiles.get(kt) or wreread_tiles.get(kt)
                nc.vector.tensor_scalar(
                    out=wt_t, in0=wt_t, scalar1=sw_b, scalar2=MAGIC,
                    op0=Alu.mult, op1=Alu.add,
                )
                nc.scalar.activation(
                    out=qwT[:, kt, :], in_=wt_t, func=Act.Sign, bias=negm
                )

            # ---- W pass 2 (the wave): qwT[kt] = sign(round(wT*sw)) as fp8,
            # one DVE op (u = w*sw + MAGIC, rounds to integer) plus one ACT
            # op. The first two row tiles' matmuls interleave with the wave
            # so each arriving qwT k-tile immediately unlocks 8 matmuls,
            # and x2/x3's quantize chains ride the wave's DVE slack (their
            # transposes issue from the now-idle sync ring).
            if MT >= 2:
                qxT0, amc0 = xq[0]
                qxT1, amc1 = xq[1]
                accs0 = [
                    pacc.tile([128, 512], f32, tag="acc", name=f"acc_0_{i}")
                    for i in range(NQ)
                ]
                accs1 = [
                    pacc.tile([128, 512], f32, tag="acc", name=f"acc_1_{i}")
                    for i in range(NQ)
                ]
                for kt in range(KT):
                    quantize_w(kt)
                    for nq in range(NQ):
                        mm(accs0[nq], qxT0, kt, nq)
                    for nq in range(NQ):
                        mm(accs1[nq], qxT1, kt, nq)
                    if kt == 2 and MT > 2:
                        x_load_gated(2)
                    if kt == 4 and MT > 2:
                        xq[2] = x_quant(2, transpose_ring=nc.sync,
                                        xs_on_dve=True)
                    if kt == 5 and MT > 3:
                        x_load_gated(3)
                    if kt == 7 and MT > 3:
                        xq[3] = x_quant(3, transpose_ring=nc.sync,
                                        xs_on_dve=True)
                finish(0, accs0, amc0)
                finish(1, accs1, amc1)
                start_mi = 2
            else:
                for kt in range(KT):
                    quantize_w(kt)
                start_mi = 0

            # ---- dense phase over the remaining row tiles
            for mi in range(start_mi, MT):
                if mi in xq:
                    qxT, amc = xq[mi]
                else:
                    qxT, amc = x_quant(mi)
                accs = [
                    pacc.tile([128, 512], f32, tag="acc", name=f"acc_{mi}_{i}")
                    for i in range(NQ)
                ]
                if mi == MT - 1:
                    # nq-inner with per-chunk dequant+store: each output
                    # chunk is dequantized and written out as soon as its
                    # 16 accumulations are done (shortest possible tail)
                    cs = small.tile([128, 1], f32, tag="small")
                    nc.vector.tensor_mul(cs, amc, q_b)
                    o_t = outp.tile([128, N], f16, tag="outp", name=f"o{mi}")
                    for nq in range(NQ):
                        for kt in range(KT):
                            mm(accs[nq], qxT, kt, nq)
                        nc.scalar.activation(
                            out=o_t[:, ts(nq, 512)], in_=accs[nq],
                            func=Act.Copy, scale=cs,
                        )
                        nc.scalar.dma_start(
                            out=out_ext[ts(mi, 128), ts(nq, 512)],
                            in_=o_t[:, ts(nq, 512)],
                        )
                else:
                    for kt in range(KT):
                        for nq in range(NQ):
                            mm(accs[nq], qxT, kt, nq)
                    finish(mi, accs, amc)

    nc.compile()
    return nc


_NC_CACHE = {}


def _get_nc(rows_per_core):
    if rows_per_core not in _NC_CACHE:
        _NC_CACHE[rows_per_core] = build(rows_per_core)
    return _NC_CACHE[rows_per_core]


_IDENT = np.eye(128, dtype=np.float32)


def _ident_bf16():
    # bf16 identity as uint16 view (ml_dtypes-free): 1.0 -> 0x3F80
    e = np.zeros((128, 128), dtype=np.uint16)
    np.fill_diagonal(e, 0x3F80)
    return e


def run(x, weight, **spmd_kwargs):
    x = np.ascontiguousarray(np.asarray(x, dtype=np.float32))
    weight = np.asarray(weight, dtype=np.float32)
    b, s, k = x.shape
    rows = b * s
    rpc = rows // N_CORES
    xr = x.reshape(rows, k)
    wt = np.ascontiguousarray(weight.T)
    ident = _ident_bf16()
    nc = _get_nc(rpc)
    in_maps = [
        {"x": xr[i * rpc : (i + 1) * rpc], "wt": wt, "ident": ident}
        for i in range(N_CORES)
    ]
    res = run_bass_kernel_spmd(
        nc, in_maps, core_ids=list(range(N_CORES)), **spmd_kwargs
    )
    out = np.concatenate(
        [res.results[i]["out"] for i in range(N_CORES)], axis=0
    )
    return out.reshape(b, s, N), res


def kernel(x, weight):
    out, _ = run(x, weight)
    return out
